# revision 10
# baseline (speedup 1.0000x reference)
"""Trainium2 Bass kernel for CoverageAttention.

Reference computation (per example b):
    proj_f = features @ W1_w + W1_b                  # (L, U)
    proj_h = hidden @ W2_w + W2_b                    # (U,)
    proj_a = past_attention[:, None] * Uf_w + Uf_b   # (L, U)
    att    = tanh(proj_f + proj_h + proj_a)          # (L, U)
    score  = att @ V_w + V_b                         # (L, 1)
    w      = softmax(score, axis=0)                  # (L, 1)
    ctx    = sum(w * features, axis=0)               # (F,)
    return (ctx, w)

Sharding: data-parallel over batch across 8 NeuronCores (B=64 -> 8 per core);
all parameters replicated.  No cross-core communication.

Per-core kernel layout (P = 128 partitions):
  - features tiles loaded HBM->SBUF with fp32->fp16 cast during the SWDGE DMA
    (natural layout [l, f], kept resident for the final context matmul).
  - Transposed tiles featT [f, l] produced on-chip with the xbar DMA transpose
    (2-byte dtype requirement is why the matmul path runs in fp16; PSUM
    accumulation stays fp32).
  - Main matmul produces att_pre^T [u, l] in PSUM; the rank-1 term
    past_attention (x) Uf_w folds in as one extra K=1 matmul into the same
    accumulation group; (proj_h + W1_b + W2_b + Uf_b)[u] folds into the
    per-partition bias operand of the ScalarE tanh.
  - score = V^T @ att^T on the PE; softmax on [1, L] rows (V_b cancels in
    softmax and is ignored); weights transposed to columns with tiny PE
    transposes; context = w^T @ features_natural on the PE.
"""

import os
import sys

for _p in ("/opt/trn_rl_repo", "/opt/pypackages"):
    if os.path.isdir(_p) and _p not in sys.path:
        sys.path.append(_p)

import numpy as np
from contextlib import ExitStack

import concourse.bass as bass
import concourse.bacc as bacc
import concourse.mybir as mybir
import concourse.tile as tile

DT = mybir.dt
AFT = mybir.ActivationFunctionType
P = 128

# Full problem config.
N_CORES = 8
B, L, F, U, H = 64, 2048, 512, 256 * 2, 256
B_CORE = B // N_CORES
LB = 512  # l-block (matmul moving free size)


def emit_coverage_attention(tc, io, cfg):
    """Emit the per-core kernel. io: dict of name -> dram AP."""
    nc = tc.nc
    b_core = cfg["b_core"]
    l_len, f_len, u_len, h_len = cfg["L"], cfg["F"], cfg["U"], cfg["H"]
    lb = cfg["LB"]
    n_lc, n_fc, n_uc, n_hc = l_len // P, f_len // P, u_len // P, h_len // P
    n_lb = l_len // lb
    lc_per_lb = lb // P

    feats, hidden, past = io["features"], io["hidden"], io["past_attention"]
    w1, w2, ufw, v = io["W1_w"], io["W2_w"], io["Uf_w"], io["V_w"]
    w1b, w2b, ufb = io["W1_b"], io["W2_b"], io["Uf_b"]
    ctx_out, attw_out = io["context"], io["attw"]

    with ExitStack() as ctx:
        const = ctx.enter_context(tc.tile_pool(name="const", bufs=1))

        # ---- constants / parameters (cast to fp16 where used by matmuls) ----
        w1_sb = []  # per fc: [128 f, U] fp16 (lhsT slices [f, u_chunk])
        for fc in range(n_fc):
            t = const.tile([P, u_len], DT.float16, tag=f"w1_{fc}")
            nc.gpsimd.dma_start(out=t[:, :], in_=w1[fc * P:(fc + 1) * P, :])
            w1_sb.append(t)
        w2_sb = []  # per hc: [128 h, U] fp16
        for hc in range(n_hc):
            t = const.tile([P, u_len], DT.float16, tag=f"w2_{hc}")
            nc.gpsimd.dma_start(out=t[:, :], in_=w2[hc * P:(hc + 1) * P, :])
            w2_sb.append(t)
        ufw_sb = const.tile([1, u_len], DT.float16, tag="ufw")
        nc.gpsimd.dma_start(out=ufw_sb[:, :], in_=ufw[:, :])
        v_sb = const.tile([P, n_uc], DT.float16, tag="v")  # v_sb[p, c] = V[c*128+p]
        nc.gpsimd.dma_start(
            out=v_sb[:, :], in_=v.rearrange("(c p) o -> p (c o)", p=P)
        )
        hid_t = []  # per hc: [128 h, b_core] fp16  (hidden^T)
        for hc in range(n_hc):
            t = const.tile([P, b_core], DT.float16, tag=f"hidT_{hc}")
            nc.gpsimd.dma_start(
                out=t[:, :],
                in_=hidden[:, hc * P:(hc + 1) * P].rearrange("b h -> h b"),
            )
            hid_t.append(t)
        # Sum of the three per-u biases, as [128, n_uc] fp32 columns.
        bvec = []
        for i, bt in enumerate((w1b, w2b, ufb)):
            t = const.tile([P, n_uc], DT.float32, tag=f"bvec_{i}")
            nc.gpsimd.dma_start(out=t[:, :], in_=bt.rearrange("(c p) -> p c", p=P))
            bvec.append(t)
        bsum = const.tile([P, n_uc], DT.float32, tag="bsum")
        nc.vector.tensor_add(bsum[:, :], bvec[0][:, :], bvec[1][:, :])
        nc.vector.tensor_add(bsum[:, :], bsum[:, :], bvec[2][:, :])

        # ---- per-(u, example) tanh bias: proj_h^T + bsum, [128, n_uc*b_core] fp32
        bias_sb = const.tile([P, n_uc * b_core], DT.float32, tag="bias_sb")
        with tc.tile_pool(name="bias_ps", bufs=1, space="PSUM") as bias_ps_pool:
            for uc in range(n_uc):
                bps = bias_ps_pool.tile([P, b_core], DT.float32, tag="bias_ps")
                for hc in range(n_hc):
                    nc.tensor.matmul(
                        bps[:, :],
                        lhsT=w2_sb[hc][:, uc * P:(uc + 1) * P],
                        rhs=hid_t[hc][:, :],
                        start=(hc == 0),
                        stop=(hc == n_hc - 1),
                    )
                # bias_sb[:, uc*b : (uc+1)*b] = bps + bsum[:, uc]
                nc.vector.tensor_scalar(
                    bias_sb[:, uc * b_core:(uc + 1) * b_core],
                    bps[:, :],
                    bsum[:, uc:uc + 1],
                    None,
                    op0=mybir.AluOpType.add,
                )

        # ---- main pools ----
        nat_pool = ctx.enter_context(tc.tile_pool(name="nat", bufs=2 * n_lc))
        scratch_pool = ctx.enter_context(
            tc.tile_pool(name="scratch", bufs=2, space="DRAM")
        )
        featT_pool = ctx.enter_context(tc.tile_pool(name="featT", bufs=2 * n_fc))
        pa_pool = ctx.enter_context(tc.tile_pool(name="pa", bufs=2))
        att_ps_pool = ctx.enter_context(
            tc.tile_pool(name="att_ps", bufs=3, space="PSUM")
        )
        attT_pool = ctx.enter_context(tc.tile_pool(name="attT", bufs=3))
        score_ps_pool = ctx.enter_context(
            tc.tile_pool(name="score_ps", bufs=2, space="PSUM")
        )
        row_pool = ctx.enter_context(tc.tile_pool(name="rows", bufs=2))
        small_pool = ctx.enter_context(tc.tile_pool(name="small", bufs=4))
        wcol_pool = ctx.enter_context(tc.tile_pool(name="wcol", bufs=2))
        ctx_ps_pool = ctx.enter_context(
            tc.tile_pool(name="ctx_ps", bufs=2, space="PSUM")
        )

        for b in [
            bb for _ in range(cfg.get("reps", 1)) for bb in range(b_core)
        ]:
            # ---- load natural fp16 tiles (cast during DMA), one per l-chunk
            nat = []
            for lc in range(n_lc):
                t = nat_pool.tile([P, f_len], DT.float16, tag="nat")
                nc.gpsimd.dma_start(
                    out=t[:, :], in_=feats[b, lc * P:(lc + 1) * P, :]
                )
                nat.append(t)
            # past_attention row for this example, fp16
            pa_row = pa_pool.tile([1, l_len], DT.float16, tag="pa_row")
            nc.gpsimd.dma_start(out=pa_row[:, :], in_=past[b:b + 1, :])

            # ---- transpose via DRAM fp16 scratch + xbar (DRAM -> SBUF):
            # write natural fp16 tiles to scratch, then one transpose per
            # f-chunk reads [L, 128] and lands [128 f, L l] in SBUF.
            scratch = scratch_pool.tile([l_len, f_len], DT.float16, tag="scratch")
            for lc in range(n_lc):
                nc.sync.dma_start(
                    out=scratch[lc * P:(lc + 1) * P, :], in_=nat[lc][:, :]
                )
            featT = []
            for fc in range(n_fc):
                t = featT_pool.tile([P, l_len], DT.float16, tag="featT")
                nc.sync.dma_start_transpose(
                    out=t[:, :], in_=scratch[:, fc * P:(fc + 1) * P]
                )
                featT.append(t)

            score_row = row_pool.tile([1, l_len], DT.float32, tag="score_row")

            # ---- att_pre^T -> tanh -> score, per (lb, uc)
            for lbi in range(n_lb):
                sc_ps = score_ps_pool.tile([1, lb], DT.float32, tag="sc_ps")
                for uc in range(n_uc):
                    att_ps = att_ps_pool.tile([P, lb], DT.float32, tag="att_ps")
                    for fc in range(n_fc):
                        nc.tensor.matmul(
                            att_ps[:, :],
                            lhsT=w1_sb[fc][:, uc * P:(uc + 1) * P],
                            rhs=featT[fc][:, lbi * lb:(lbi + 1) * lb],
                            start=(fc == 0),
                            stop=False,
                        )
                    nc.tensor.matmul(
                        att_ps[:, :],
                        lhsT=ufw_sb[:, uc * P:(uc + 1) * P],
                        rhs=pa_row[:, lbi * lb:(lbi + 1) * lb],
                        start=False,
                        stop=True,
                    )
                    att_sb = attT_pool.tile([P, lb], DT.float16, tag="att_sb")
                    nc.scalar.activation(
                        att_sb[:, :],
                        att_ps[:, :],
                        AFT.Tanh,
                        bias=bias_sb[:, uc * b_core + b:uc * b_core + b + 1],
                    )
                    nc.tensor.matmul(
                        sc_ps[:, :],
                        lhsT=v_sb[:, uc:uc + 1],
                        rhs=att_sb[:, :],
                        start=(uc == 0),
                        stop=(uc == n_uc - 1),
                    )
                nc.vector.tensor_copy(
                    score_row[:, lbi * lb:(lbi + 1) * lb], sc_ps[:, :]
                )

            # ---- softmax over l (row layout)
            neg_max = small_pool.tile([1, 1], DT.float32, tag="neg_max")
            nc.vector.tensor_reduce(
                neg_max[:, :],
                score_row[:, :],
                axis=mybir.AxisListType.X,
                op=mybir.AluOpType.max,
                negate=True,
            )
            exp_row = row_pool.tile([1, l_len], DT.float32, tag="exp_row")
            esum = small_pool.tile([1, 1], DT.float32, tag="esum")
            nc.scalar.activation(
                exp_row[:, :],
                score_row[:, :],
                AFT.Exp,
                bias=neg_max[:, :],
                accum_out=esum[:, :],
            )
            recip = small_pool.tile([1, 1], DT.float32, tag="recip")
            nc.vector.reciprocal(recip[:, :], esum[:, :])
            w_row = row_pool.tile([1, l_len], DT.float32, tag="w_row")
            nc.vector.tensor_scalar_mul(w_row[:, :], exp_row[:, :], recip[:, :])
            nc.sync.dma_start(out=attw_out[b:b + 1, :], in_=w_row[:, :])

            # ---- w columns [128, n_lc] fp16: bounce through DRAM, then the
            # proven DRAM->SBUF rearrange+cast DMA pattern.
            w_scr = scratch_pool.tile([1, l_len], DT.float32, tag="w_scr")
            nc.sync.dma_start(out=w_scr[:, :], in_=w_row[:, :])
            w_col = wcol_pool.tile([P, n_lc], DT.float16, tag="w_col")
            nc.gpsimd.dma_start(
                out=w_col[:, :],
                in_=w_scr.rearrange("one (j p) -> p (one j)", p=P),
            )

            # ---- context = sum_l w[l] * features[l, :]
            ctx_ps = ctx_ps_pool.tile([1, f_len], DT.float32, tag="ctx_ps")
            for lc in range(n_lc):
                nc.tensor.matmul(
                    ctx_ps[:, :],
                    lhsT=w_col[:, lc:lc + 1],
                    rhs=nat[lc][:, :],
                    start=(lc == 0),
                    stop=(lc == n_lc - 1),
                )
            ctx_row = row_pool.tile([1, f_len], DT.float32, tag="ctx_row")
            nc.vector.tensor_copy(ctx_row[:, :], ctx_ps[:, :])
            nc.sync.dma_start(out=ctx_out[b:b + 1, :], in_=ctx_row[:, :])


def build_module(cfg):
    """Build + compile the per-core Bass module. Returns (nc, io_names)."""
    nc = bacc.Bacc("TRN2", target_bir_lowering=False, debug=False)
    b_core = cfg["b_core"]
    l_len, f_len, u_len, h_len = cfg["L"], cfg["F"], cfg["U"], cfg["H"]
    io = {
        "features": nc.dram_tensor(
            "features", [b_core, l_len, f_len], DT.float32, kind="ExternalInput"
        ).ap(),
        "hidden": nc.dram_tensor(
            "hidden", [b_core, h_len], DT.float32, kind="ExternalInput"
        ).ap(),
        "past_attention": nc.dram_tensor(
            "past_attention", [b_core, l_len], DT.float32, kind="ExternalInput"
        ).ap(),
        "W1_w": nc.dram_tensor(
            "W1_w", [f_len, u_len], DT.float32, kind="ExternalInput"
        ).ap(),
        "W2_w": nc.dram_tensor(
            "W2_w", [h_len, u_len], DT.float32, kind="ExternalInput"
        ).ap(),
        "Uf_w": nc.dram_tensor(
            "Uf_w", [1, u_len], DT.float32, kind="ExternalInput"
        ).ap(),
        "W1_b": nc.dram_tensor(
            "W1_b", [u_len], DT.float32, kind="ExternalInput"
        ).ap(),
        "W2_b": nc.dram_tensor(
            "W2_b", [u_len], DT.float32, kind="ExternalInput"
        ).ap(),
        "Uf_b": nc.dram_tensor(
            "Uf_b", [u_len], DT.float32, kind="ExternalInput"
        ).ap(),
        "V_w": nc.dram_tensor(
            "V_w", [u_len, 1], DT.float32, kind="ExternalInput"
        ).ap(),
        "context": nc.dram_tensor(
            "context", [b_core, f_len], DT.float32, kind="ExternalOutput"
        ).ap(),
        "attw": nc.dram_tensor(
            "attw", [b_core, l_len], DT.float32, kind="ExternalOutput"
        ).ap(),
    }
    with tile.TileContext(nc) as tc:
        emit_coverage_attention(tc, io, cfg)
    nc.compile()
    return nc


_NC_CACHE = {}


def _get_module(reps=1):
    if reps not in _NC_CACHE:
        cfg = dict(b_core=B_CORE, L=L, F=F, U=U, H=H, LB=LB, reps=reps)
        _NC_CACHE[reps] = build_module(cfg)
    return _NC_CACHE[reps]


def run_kernel_spmd(inputs, trace=False, trace_kwargs=None, reps=1):
    """Shard inputs over 8 cores, run, gather. Returns (outputs, results)."""
    from concourse.bass_utils import run_bass_kernel_spmd

    f32 = lambda x: np.ascontiguousarray(np.asarray(x), dtype=np.float32)
    features = f32(inputs["features"])
    hidden = f32(inputs["hidden"])
    past = f32(inputs["past_attention"])
    w1, w1b = f32(inputs["W1_w"]), f32(inputs["W1_b"])
    w2, w2b = f32(inputs["W2_w"]), f32(inputs["W2_b"])
    ufw, ufb = f32(inputs["Uf_w"]), f32(inputs["Uf_b"])
    vw = f32(inputs["V_w"])

    nc = _get_module(reps)
    in_maps = []
    for c in range(N_CORES):
        s = slice(c * B_CORE, (c + 1) * B_CORE)
        in_maps.append(
            {
                "features": features[s],
                "hidden": hidden[s],
                "past_attention": past[s],
                "W1_w": w1,
                "W2_w": w2,
                "Uf_w": ufw,
                "W1_b": w1b,
                "W2_b": w2b,
                "Uf_b": ufb,
                "V_w": vw,
            }
        )
    res = run_bass_kernel_spmd(
        nc,
        in_maps,
        core_ids=list(range(N_CORES)),
        trace=trace,
        trace_kwargs=trace_kwargs or {},
    )
    ctx = np.concatenate([r["context"] for r in res.results], axis=0)
    attw = np.concatenate([r["attw"] for r in res.results], axis=0)
    return (ctx, attw.reshape(B, L, 1)), res


def kernel(**inputs):
    outputs, _ = run_kernel_spmd(inputs, trace=False)
    return outputs
